# revision 13
# baseline (speedup 1.0000x reference)
"""Multi-head attention (B=8, N=1024, C=768, H=12) on 8 TRN2 NeuronCores.

Sharding: data-parallel over the batch - core i computes batch element i.
No collectives.

Per-core math (feature-major, no on-device transposes):
  qkT   = w_qkv[:, :1536].T @ xT            # [1536, 1024]
  v_tok = xT.T @ w_qkv[:, 1536:]            # [1024, 768] token-major + ones col
  per head h:
    ST   = k_h @ q_h^T                      # [1024k, 1024q]  K=64 matmuls;
                                            #   head pairs issue adjacently at PE
                                            #   tile positions (0,0)/(64,0) and run
                                            #   row-group-concurrent (~227ns/pair)
    E    = exp(SCALE * ST)                  # bf16, no max-subtraction
    [O_un; d] = [v_h | 1].T @ E             # [65, 512]: row 64 = softmax denom
    norm: dd <- d (partition-shift copy), rr = recip(dd), rbc = gpsimd
          broadcast, ot <- O_un * rbc       # fused normalize+evict on DVE
  yT = w_proj.T @ OT;  y += b via ACT scalar.add at psum eviction

Schedule: 6 phases (one per head pair), scores+exp streaming with qk/v/PV/
norm fillers; projection split into pre-opened chunks + tail closes.
ACT runs exp-only mid-stream (table preloaded by a warmup exp during the
DMA head); weights are host-packed per pair ([6, 128, KT*256]) so the
critical path x + pair-0 cols lands in a few big-line descriptors across
both hwdge queues (sync + scalar).

Host side: kernel() takes full inputs, pre-transposes/casts/packs, runs one
SPMD NEFF on 8 cores, re-transposes/stacks outputs. HW exec ~187us
(neuron-profile exec_time_ns), rel err ~5.3e-3.
"""

import os
import sys

import numpy as np

for _p in ("/opt/trn_rl_repo", "/root/.axon_site/_ro/trn_rl_repo"):
    if os.path.isdir(_p) and _p not in sys.path:
        sys.path.insert(0, _p)

import concourse.bacc as bacc
import concourse.mybir as mybir
import concourse.tile as tile

F32 = mybir.dt.float32
BF16 = mybir.dt.bfloat16

B, NT, C = 8, 1024, 768
H, HD = 12, 64
C3 = 3 * C          # 2304
SCALE = HD ** -0.5  # 0.125
KT = C // 128       # 6   k-tiles over the C contraction
MQK = 1536 // 128   # 12  row-blocks of qkT
TT = NT // 128      # 8   token tiles
NQ = NT // 512      # 2   512-wide q slices
VA = HD + 1         # 65  v columns per head + ones column


def build_graph(tc):
    nc = tc.nc
    xt_d = nc.dram_tensor("xT", [C, NT], BF16, kind="ExternalInput").ap()
    wqkp_d = nc.dram_tensor("wqkp", [6, 128, KT * 256], BF16, kind="ExternalInput").ap()
    wv_d = nc.dram_tensor("wv", [C, C], BF16, kind="ExternalInput").ap()
    wproj_d = nc.dram_tensor("wproj", [C, C], BF16, kind="ExternalInput").ap()
    bp_d = nc.dram_tensor("bproj", [128, KT], F32, kind="ExternalInput").ap()
    out_d = nc.dram_tensor("out", [C, NT], F32, kind="ExternalOutput").ap()

    from contextlib import ExitStack

    with ExitStack() as stack:
        persist = stack.enter_context(tc.tile_pool(name="persist", bufs=1))
        qk_sb = persist.tile([128, MQK * NT], BF16)      # qkT feature-major
        vaug = persist.tile([128, TT * H * VA], BF16)    # [v_h | 1] per head, token-major
        ot03 = persist.tile([128, 4 * NT], BF16)         # attention out blocks 0-3
        ot4 = persist.tile([128, NT], BF16)              # block 4 (heads 8/9)
        ot5 = persist.tile([128, NT], BF16)              # block 5 (heads 10/11)

        def ot_ap(blk, p0, p1, c0, c1):
            if blk < 4:
                return ot03[p0:p1, blk * NT + c0 : blk * NT + c1]
            t = ot4 if blk == 4 else ot5
            return t[p0:p1, c0:c1]

        wqp_sb = persist.tile([128, 6 * KT * 256], BF16)   # packed q/k cols per pair
        wv_sb = persist.tile([128, KT * C], BF16)          # v cols
        xt_sb = persist.tile([128, KT * NT], BF16)
        wp_sb = persist.tile([128, KT * C], BF16)
        bp_sb = persist.tile([128, KT], F32)

        warm_in = persist.tile([1, 8], F32)
        warm_out = persist.tile([1, 8], BF16)

        attn = stack.enter_context(tc.tile_pool(name="attn", bufs=1))
        ps_st = stack.enter_context(tc.tile_pool(name="ps_st", bufs=2, space="PSUM"))
        ps_pv = stack.enter_context(tc.tile_pool(name="ps_pv", bufs=2, space="PSUM"))
        ps = stack.enter_context(tc.tile_pool(name="ps", bufs=2, space="PSUM"))

        # ---- constants ----
        nc.vector.memset(warm_in[:, :], 0.0)
        nc.vector.memset(
            vaug[:, :].rearrange("p (g c) -> p g c", g=TT * H, c=VA)[:, :, HD : HD + 1],
            1.0,
        )

        # ---- critical DMAs: host-packed pair-0 q/k cols (one big-line
        #      descriptor) + x tiles, interleaved across the two hwdge queues ----
        def dma_qk_cols(b, eng=None):
            (eng or nc.sync).dma_start(
                out=wqp_sb[:, b * KT * 256 : (b + 1) * KT * 256],
                in_=wqkp_d[b, :, :],
            )

        dma_qk_cols(0, nc.scalar)
        for k in range(KT):
            eng = nc.sync if k % 2 == 0 else nc.scalar
            eng.dma_start(
                out=xt_sb[:, k * NT : (k + 1) * NT],
                in_=xt_d[k * 128 : (k + 1) * 128, :],
            )

        # ---- ACT exp-table preload (concurrent with DMAs) ----
        nc.scalar.activation(warm_out[:, :], warm_in[:, :],
                             mybir.ActivationFunctionType.Exp, scale=SCALE)

        def dma_v_cols():
            for k in range(KT):
                eng = nc.sync if k % 2 == 0 else nc.scalar
                eng.dma_start(
                    out=wv_sb[:, k * C : (k + 1) * C],
                    in_=wv_d[k * 128 : (k + 1) * 128, :],
                )

        def dma_wproj():
            for k in range(KT):
                nc.sync.dma_start(
                    out=wp_sb[:, k * C : (k + 1) * C],
                    in_=wproj_d[k * 128 : (k + 1) * 128, :],
                )
            nc.sync.dma_start(out=bp_sb[:, :], in_=bp_d[:, :])

        def emit_v_group(t, j):
            psv = ps.tile([128, 384], F32, name=f"psv{t}_{j}", tag="ps")
            for k in range(KT):
                nc.tensor.matmul(
                    psv[:, :],
                    xt_sb[:, k * NT + t * 128 : k * NT + (t + 1) * 128],
                    wv_sb[:, k * C + j * 384 : k * C + (j + 1) * 384],
                    start=(k == 0),
                    stop=(k == KT - 1),
                )
            h0 = 6 * j
            nc.vector.tensor_copy(
                vaug[:, t * H * VA + h0 * VA : t * H * VA + (h0 + 6) * VA]
                .rearrange("p (g c) -> p g c", g=6, c=VA)[:, :, 0:HD],
                psv[:, :].rearrange("p (g c) -> p g c", g=6, c=HD),
            )

        def emit_qk_group(m, n):
            b, half = m % 6, (0 if m < 6 else 1)
            psq = ps.tile([128, 512], F32, name=f"psq{m}_{n}", tag="ps")
            for k in range(KT):
                nc.tensor.matmul(
                    psq[:, :],
                    wqp_sb[:, b * KT * 256 + k * 256 + half * 128 :
                           b * KT * 256 + k * 256 + half * 128 + 128],
                    xt_sb[:, k * NT + n * 512 : k * NT + (n + 1) * 512],
                    start=(k == 0),
                    stop=(k == KT - 1),
                )
            nc.vector.tensor_copy(
                qk_sb[:, m * NT + n * 512 : m * NT + n * 512 + 512], psq[:, :]
            )

        # expst pair layout: pair p holds heads (2p, 2p+1);
        # slice for (h, kt, qs) = [:, kt*2048 + (h%2)*1024 + qs*512 :][:512]
        pair_tiles = {}

        def emit_st_pair_kt(p, kt):
            """Scores for both heads of pair p, k-token-tile kt. The hp0/hp1
            matmuls have lhsT/rhs at partition bases 0/64 and land at PE tile
            positions (0,0)/(64,0) -> concurrent row-group execution."""
            ep = pair_tiles[p]
            tt = [
                ps_st.tile([128, 1024], F32, name=f"st{p}_{kt}_{hp}", tag="st")
                for hp in range(2)
            ]
            for qs in range(NQ):
                for hp in range(2):
                    p0 = hp * 64
                    nc.tensor.matmul(
                        tt[hp][:, qs * 512 : qs * 512 + 512],
                        qk_sb[p0 : p0 + 64,
                              (6 + p) * NT + kt * 128 : (6 + p) * NT + (kt + 1) * 128],
                        qk_sb[p0 : p0 + 64,
                              p * NT + qs * 512 : p * NT + (qs + 1) * 512],
                        start=True,
                        stop=True,
                    )
            for hp in range(2):
                nc.scalar.activation(
                    ep[:, kt * 2048 + hp * 1024 : kt * 2048 + hp * 1024 + 1024],
                    tt[hp][:, :],
                    mybir.ActivationFunctionType.Exp,
                    scale=SCALE,
                )

        def emit_pv(h, qs):
            ep = pair_tiles[h // 2]
            pso = ps_pv.tile([VA, 512], F32, name=f"pso{h}_{qs}", tag="pv")
            for kt in range(TT):
                nc.tensor.matmul(
                    pso[:, :],
                    vaug[:, kt * H * VA + h * VA : kt * H * VA + (h + 1) * VA],
                    ep[:, kt * 2048 + (h % 2) * 1024 + qs * 512 :
                       kt * 2048 + (h % 2) * 1024 + qs * 512 + 512],
                    start=(kt == 0),
                    stop=(kt == TT - 1),
                )
            return pso

        def emit_norm(h, qs, pso):
            """denominator -> reciprocal -> broadcast -> fused normalize+evict."""
            p0 = (h % 2) * 64
            qblk = h // 2
            with tc.high_priority():
                dd = attn.tile([1, 512], F32, name=f"dd{h}_{qs}", tag="dd", bufs=3)
                nc.vector.tensor_copy(dd[0:1, :], pso[64:65, :])
                rr = attn.tile([1, 512], F32, name=f"rr{h}_{qs}", tag="rr", bufs=3)
                nc.vector.reciprocal_approx_fast(out=rr[0:1, :], in_=dd[0:1, :])
                rbc = attn.tile([128, 512], F32, name=f"rbc{h}_{qs}", tag="rbc", bufs=3)
                nc.gpsimd.partition_broadcast(rbc[:, :], rr[0:1, :])
                nc.vector.tensor_mul(
                    ot_ap(qblk, p0, p0 + 64, qs * 512, qs * 512 + 512),
                    pso[0:64, :],
                    rbc[p0 : p0 + 64, :],
                )

        def emit_pv_norm(h, qs):
            emit_norm(h, qs, emit_pv(h, qs))

        def emit_proj_open(m, ns, kmax, pool=None):
            pl = pool if pool is not None else ps
            psy = pl.tile([128, 512], F32, name=f"psy{m}_{ns}",
                          tag="st" if pl is ps_st else "ps")
            for k in range(kmax):
                nc.tensor.matmul(
                    psy[:, :],
                    wp_sb[:, k * C + m * 128 : k * C + (m + 1) * 128],
                    ot_ap(k, 0, 128, ns * 512, (ns + 1) * 512),
                    start=(k == 0),
                    stop=False,
                )
            return psy

        def emit_proj_open(m, ns, kmax, pool=None):
            pl = pool if pool is not None else ps
            psy = pl.tile([128, 512], F32, name=f"psy{m}_{ns}",
                          tag="st" if pl is ps_st else "ps")
            for k in range(kmax):
                nc.tensor.matmul(
                    psy[:, :],
                    wp_sb[:, k * C + m * 128 : k * C + (m + 1) * 128],
                    ot_ap(k, 0, 128, ns * 512, (ns + 1) * 512),
                    start=(k == 0),
                    stop=False,
                )
            return psy

        def emit_proj_close(psy, m, ns, kmin):
            for k in range(kmin, KT):
                nc.tensor.matmul(
                    psy[:, :],
                    wp_sb[:, k * C + m * 128 : k * C + (m + 1) * 128],
                    ot_ap(k, 0, 128, ns * 512, (ns + 1) * 512),
                    start=False,
                    stop=(k == KT - 1),
                )
            yt = attn.tile([128, 512], F32, name=f"yt{m}_{ns}", tag="yt", bufs=3)
            nc.scalar.add(yt[:, :], psy[:, :], bp_sb[:, m : m + 1])
            nc.sync.dma_start(
                out=out_d[m * 128 : (m + 1) * 128, ns * 512 : (ns + 1) * 512],
                in_=yt[:, :],
            )

        def emit_proj(m, ns):
            emit_proj_close(emit_proj_open(m, ns, 5), m, ns, 5)

        # ---- prologue: qk for pair 0 (DMA-paced), then deferred DMAs ----
        for n in range(NQ):
            emit_qk_group(6, n)
            emit_qk_group(0, n)
        dma_v_cols()

        # ---- main loop over head pairs ----
        held = {}
        for b in range(6):
            fillers = []
            if b == 0:
                fillers.append(lambda: dma_qk_cols(1))
                for t in range(TT):
                    fillers.append(lambda t=t: (emit_v_group(t, 0),
                                                emit_v_group(t, 1)))
                fillers.insert(3, lambda: (emit_qk_group(1, 0),
                                           emit_qk_group(1, 1)))
                fillers.insert(6, lambda: (emit_qk_group(7, 0),
                                           emit_qk_group(7, 1)))
                fillers.append(dma_wproj)
            elif b < 5:
                fillers.append(lambda b=b: dma_qk_cols(b + 1))
                fillers.append(lambda b=b: (emit_qk_group(b + 1, 0),
                                            emit_qk_group(b + 1, 1)))
                fillers.append(lambda b=b: (emit_pv_norm(2 * b - 2, 0),
                                            emit_pv_norm(2 * b - 1, 0)))
                fillers.append(lambda b=b: (emit_qk_group(7 + b, 0),
                                            emit_qk_group(7 + b, 1)))
                fillers.append(lambda b=b: (emit_pv_norm(2 * b - 2, 1),
                                            emit_pv_norm(2 * b - 1, 1)))
            else:
                fillers.append(lambda: (emit_pv_norm(8, 0), emit_pv_norm(9, 0)))
                fillers.append(lambda: held.update(g00=emit_proj_open(0, 0, 4)))
                fillers.append(lambda: (emit_pv_norm(8, 1), emit_pv_norm(9, 1)))
                fillers.append(lambda: held.update(g10=emit_proj_open(1, 0, 4)))
            pair_tiles[b] = attn.tile([128, TT * 2048], BF16, name=f"epair{b}",
                                      tag="epair", bufs=2)
            fi = 0
            for kt in range(TT):
                emit_st_pair_kt(b, kt)
                if fi < len(fillers):
                    fillers[fi]()
                    fi += 1
            while fi < len(fillers):
                fillers[fi]()
                fi += 1

        # ---- tail: PV/norm heads 10/11 interleaved with projection ----
        emit_pv_norm(10, 0)
        emit_pv_norm(11, 0)
        held["g20"] = emit_proj_open(2, 0, 5, pool=ps_st)
        emit_pv_norm(10, 1)
        held["g30"] = emit_proj_open(3, 0, 5, pool=ps_st)
        emit_pv_norm(11, 1)
        emit_proj_close(held["g00"], 0, 0, 4)
        emit_proj_close(held["g10"], 1, 0, 4)
        emit_proj_close(held["g20"], 2, 0, 5)
        emit_proj_close(held["g30"], 3, 0, 5)
        emit_proj(4, 0)
        emit_proj(5, 0)
        for m in range(KT):
            emit_proj(m, 1)


_NC = None


def build_nc():
    global _NC
    if _NC is None:
        nc = bacc.Bacc(
            trn_type="TRN2",
            target_bir_lowering=False,
            debug=False,
            enable_asserts=False,
            num_devices=8,
        )
        with tile.TileContext(nc) as tc:
            build_graph(tc)
        nc.compile()
        _NC = nc
    return _NC


def make_in_maps(x, w_qkv, w_proj, b_proj):
    import ml_dtypes

    bf16 = ml_dtypes.bfloat16
    x = np.asarray(x, dtype=np.float32)
    w_qkv = np.asarray(w_qkv, dtype=np.float32).astype(bf16)
    w_proj = np.ascontiguousarray(np.asarray(w_proj, dtype=np.float32).astype(bf16))
    b_proj = np.asarray(b_proj, dtype=np.float32)
    xT = np.ascontiguousarray(x.transpose(0, 2, 1).astype(bf16))  # [8, 768, 1024]
    # per-pair packed q/k col blocks in SBUF layout: [6, 128, KT*256]
    wqkp = np.empty((6, 128, KT * 256), dtype=bf16)
    for b in range(6):
        blk = np.concatenate(
            [w_qkv[:, b * 128 : (b + 1) * 128],
             w_qkv[:, 768 + b * 128 : 768 + (b + 1) * 128]], axis=1)  # [768, 256]
        wqkp[b] = np.ascontiguousarray(
            blk.reshape(KT, 128, 256).transpose(1, 0, 2).reshape(128, KT * 256))
    wv = np.ascontiguousarray(w_qkv[:, 1536:])                    # [768, 768]
    bp = np.ascontiguousarray(b_proj.reshape(KT, 128).T)          # [128, 6]
    return [
        {"xT": xT[i], "wqkp": wqkp, "wv": wv, "wproj": w_proj, "bproj": bp}
        for i in range(B)
    ]


def run_on_hw(in_maps, trace=False, **kwargs):
    from concourse.bass_utils import run_bass_kernel_spmd

    nc = build_nc()
    return run_bass_kernel_spmd(
        nc, in_maps, core_ids=list(range(B)), trace=trace, **kwargs
    )


def kernel(x, w_qkv, w_proj, b_proj):
    in_maps = make_in_maps(x, w_qkv, w_proj, b_proj)
    res = run_on_hw(in_maps, trace=False)
    out = np.stack([np.asarray(res.results[i]["out"]).T for i in range(B)])
    return np.ascontiguousarray(out.astype(np.float32))


# revision 14
# speedup vs baseline: 1.0031x; 1.0031x over previous
"""Multi-head attention (B=8, N=1024, C=768, H=12) on 8 TRN2 NeuronCores.

Sharding: data-parallel over the batch - core i computes batch element i.
No collectives.

Per-core math (feature-major, no on-device transposes):
  qkT   = w_qkv[:, :1536].T @ xT            # [1536, 1024]
  v_tok = xT.T @ w_qkv[:, 1536:]            # [1024, 768] token-major + ones col
  per head h:
    ST   = k_h @ q_h^T                      # [1024k, 1024q]  K=64 matmuls;
                                            #   head pairs issue adjacently at PE
                                            #   tile positions (0,0)/(64,0) and run
                                            #   row-group-concurrent (~227ns/pair)
    E    = exp(SCALE * ST)                  # bf16, no max-subtraction
    [O_un; d] = [v_h | 1].T @ E             # [65, 512]: row 64 = softmax denom
    norm: dd <- d (partition-shift copy), rr = recip(dd), rbc = gpsimd
          broadcast, ot <- O_un * rbc       # fused normalize+evict on DVE
  yT = w_proj.T @ OT;  y += b via ACT scalar.add at psum eviction

Schedule: 6 phases (one per head pair), scores+exp streaming with qk/v/PV/
norm fillers; projection split into pre-opened chunks + tail closes.
ACT runs exp-only mid-stream (table preloaded by a warmup exp during the
DMA head); weights are host-packed per pair ([6, 128, KT*256]) so the
critical path x + pair-0 cols lands in a few big-line descriptors across
both hwdge queues (sync + scalar).

Host side: kernel() takes full inputs, pre-transposes/casts/packs, runs one
SPMD NEFF on 8 cores, re-transposes/stacks outputs. HW exec ~187us
(neuron-profile exec_time_ns), rel err ~5.3e-3.
"""

import os
import sys

import numpy as np

for _p in ("/opt/trn_rl_repo", "/root/.axon_site/_ro/trn_rl_repo"):
    if os.path.isdir(_p) and _p not in sys.path:
        sys.path.insert(0, _p)

import concourse.bacc as bacc
import concourse.mybir as mybir
import concourse.tile as tile

F32 = mybir.dt.float32
BF16 = mybir.dt.bfloat16

B, NT, C = 8, 1024, 768
H, HD = 12, 64
C3 = 3 * C          # 2304
SCALE = HD ** -0.5  # 0.125
KT = C // 128       # 6   k-tiles over the C contraction
MQK = 1536 // 128   # 12  row-blocks of qkT
TT = NT // 128      # 8   token tiles
NQ = NT // 512      # 2   512-wide q slices
VA = HD + 1         # 65  v columns per head + ones column


def build_graph(tc):
    nc = tc.nc
    xt_d = nc.dram_tensor("xT", [C, NT], BF16, kind="ExternalInput").ap()
    wqkp_d = nc.dram_tensor("wqkp", [6, 128, KT * 256], BF16, kind="ExternalInput").ap()
    wv_d = nc.dram_tensor("wv", [C, C], BF16, kind="ExternalInput").ap()
    wproj_d = nc.dram_tensor("wproj", [C, C], BF16, kind="ExternalInput").ap()
    bp_d = nc.dram_tensor("bproj", [128, KT], F32, kind="ExternalInput").ap()
    out_d = nc.dram_tensor("out", [C, NT], F32, kind="ExternalOutput").ap()

    from contextlib import ExitStack

    with ExitStack() as stack:
        persist = stack.enter_context(tc.tile_pool(name="persist", bufs=1))
        qk_sb = persist.tile([128, MQK * NT], BF16)      # qkT feature-major
        vaug = persist.tile([128, TT * H * VA], BF16)    # [v_h | 1] per head, token-major
        ot03 = persist.tile([128, 4 * NT], BF16)         # attention out blocks 0-3
        ot4 = persist.tile([128, NT], BF16)              # block 4 (heads 8/9)
        ot5 = persist.tile([128, NT], BF16)              # block 5 (heads 10/11)

        def ot_ap(blk, p0, p1, c0, c1):
            if blk < 4:
                return ot03[p0:p1, blk * NT + c0 : blk * NT + c1]
            t = ot4 if blk == 4 else ot5
            return t[p0:p1, c0:c1]

        wqp_sb = persist.tile([128, 6 * KT * 256], BF16)   # packed q/k cols per pair
        wv_sb = persist.tile([128, KT * C], BF16)          # v cols
        xt_sb = persist.tile([128, KT * NT], BF16)
        wp_sb = persist.tile([128, KT * C], BF16)
        bp_sb = persist.tile([128, KT], F32)

        warm_in = persist.tile([1, 8], F32)
        warm_out = persist.tile([1, 8], BF16)

        attn = stack.enter_context(tc.tile_pool(name="attn", bufs=1))
        ps_st = stack.enter_context(tc.tile_pool(name="ps_st", bufs=2, space="PSUM"))
        ps_pv = stack.enter_context(tc.tile_pool(name="ps_pv", bufs=2, space="PSUM"))
        ps = stack.enter_context(tc.tile_pool(name="ps", bufs=2, space="PSUM"))

        # ---- constants ----
        nc.vector.memset(warm_in[:, :], 0.0)
        nc.vector.memset(
            vaug[:, :].rearrange("p (g c) -> p g c", g=TT * H, c=VA)[:, :, HD : HD + 1],
            1.0,
        )

        # ---- critical DMAs: host-packed pair-0 q/k cols (one big-line
        #      descriptor) + x tiles, interleaved across the two hwdge queues ----
        def dma_qk_cols(b, eng=None):
            (eng or nc.sync).dma_start(
                out=wqp_sb[:, b * KT * 256 : (b + 1) * KT * 256],
                in_=wqkp_d[b, :, :],
            )

        dma_qk_cols(0, nc.scalar)
        for k in range(KT):
            eng = nc.sync if k % 2 == 0 else nc.scalar
            eng.dma_start(
                out=xt_sb[:, k * NT : (k + 1) * NT],
                in_=xt_d[k * 128 : (k + 1) * 128, :],
            )

        # ---- ACT exp-table preload (concurrent with DMAs) ----
        nc.scalar.activation(warm_out[:, :], warm_in[:, :],
                             mybir.ActivationFunctionType.Exp, scale=SCALE)

        def dma_v_cols():
            for k in range(KT):
                eng = nc.sync if k % 2 == 0 else nc.scalar
                eng.dma_start(
                    out=wv_sb[:, k * C : (k + 1) * C],
                    in_=wv_d[k * 128 : (k + 1) * 128, :],
                )

        def dma_wproj():
            for k in range(KT):
                nc.sync.dma_start(
                    out=wp_sb[:, k * C : (k + 1) * C],
                    in_=wproj_d[k * 128 : (k + 1) * 128, :],
                )
            nc.sync.dma_start(out=bp_sb[:, :], in_=bp_d[:, :])

        def emit_v_group(t, j):
            psv = ps.tile([128, 384], F32, name=f"psv{t}_{j}", tag="ps")
            for k in range(KT):
                nc.tensor.matmul(
                    psv[:, :],
                    xt_sb[:, k * NT + t * 128 : k * NT + (t + 1) * 128],
                    wv_sb[:, k * C + j * 384 : k * C + (j + 1) * 384],
                    start=(k == 0),
                    stop=(k == KT - 1),
                )
            h0 = 6 * j
            nc.vector.tensor_copy(
                vaug[:, t * H * VA + h0 * VA : t * H * VA + (h0 + 6) * VA]
                .rearrange("p (g c) -> p g c", g=6, c=VA)[:, :, 0:HD],
                psv[:, :].rearrange("p (g c) -> p g c", g=6, c=HD),
            )

        def emit_qk_group(m, n):
            b, half = m % 6, (0 if m < 6 else 1)
            psq = ps.tile([128, 512], F32, name=f"psq{m}_{n}", tag="ps")
            for k in range(KT):
                nc.tensor.matmul(
                    psq[:, :],
                    wqp_sb[:, b * KT * 256 + k * 256 + half * 128 :
                           b * KT * 256 + k * 256 + half * 128 + 128],
                    xt_sb[:, k * NT + n * 512 : k * NT + (n + 1) * 512],
                    start=(k == 0),
                    stop=(k == KT - 1),
                )
            nc.vector.tensor_copy(
                qk_sb[:, m * NT + n * 512 : m * NT + n * 512 + 512], psq[:, :]
            )

        # expst pair layout: pair p holds heads (2p, 2p+1);
        # slice for (h, kt, qs) = [:, kt*2048 + (h%2)*1024 + qs*512 :][:512]
        pair_tiles = {}

        def emit_st_pair_kt(p, kt):
            """Scores for both heads of pair p, k-token-tile kt. The hp0/hp1
            matmuls have lhsT/rhs at partition bases 0/64 and land at PE tile
            positions (0,0)/(64,0) -> concurrent row-group execution."""
            ep = pair_tiles[p]
            tt = [
                ps_st.tile([128, 1024], F32, name=f"st{p}_{kt}_{hp}", tag="st")
                for hp in range(2)
            ]
            for qs in range(NQ):
                for hp in range(2):
                    p0 = hp * 64
                    nc.tensor.matmul(
                        tt[hp][:, qs * 512 : qs * 512 + 512],
                        qk_sb[p0 : p0 + 64,
                              (6 + p) * NT + kt * 128 : (6 + p) * NT + (kt + 1) * 128],
                        qk_sb[p0 : p0 + 64,
                              p * NT + qs * 512 : p * NT + (qs + 1) * 512],
                        start=True,
                        stop=True,
                    )
            for hp in range(2):
                nc.scalar.activation(
                    ep[:, kt * 2048 + hp * 1024 : kt * 2048 + hp * 1024 + 1024],
                    tt[hp][:, :],
                    mybir.ActivationFunctionType.Exp,
                    scale=SCALE,
                )

        def emit_pv(h, qs):
            ep = pair_tiles[h // 2]
            pso = ps_pv.tile([VA, 512], F32, name=f"pso{h}_{qs}", tag="pv")
            for kt in range(TT):
                nc.tensor.matmul(
                    pso[:, :],
                    vaug[:, kt * H * VA + h * VA : kt * H * VA + (h + 1) * VA],
                    ep[:, kt * 2048 + (h % 2) * 1024 + qs * 512 :
                       kt * 2048 + (h % 2) * 1024 + qs * 512 + 512],
                    start=(kt == 0),
                    stop=(kt == TT - 1),
                )
            return pso

        def emit_norm(h, qs, pso):
            """denominator -> reciprocal -> broadcast -> fused normalize+evict."""
            p0 = (h % 2) * 64
            qblk = h // 2
            with tc.high_priority():
                dd = attn.tile([1, 512], F32, name=f"dd{h}_{qs}", tag="dd", bufs=3)
                nc.vector.tensor_copy(dd[0:1, :], pso[64:65, :])
                rr = attn.tile([1, 512], F32, name=f"rr{h}_{qs}", tag="rr", bufs=3)
                nc.vector.reciprocal_approx_fast(out=rr[0:1, :], in_=dd[0:1, :])
                rbc = attn.tile([128, 512], F32, name=f"rbc{h}_{qs}", tag="rbc", bufs=3)
                nc.gpsimd.partition_broadcast(rbc[:, :], rr[0:1, :])
                nc.vector.tensor_mul(
                    ot_ap(qblk, p0, p0 + 64, qs * 512, qs * 512 + 512),
                    pso[0:64, :],
                    rbc[p0 : p0 + 64, :],
                )

        def emit_pv_norm(h, qs):
            emit_norm(h, qs, emit_pv(h, qs))

        def emit_proj_open(m, ns, kmax, pool=None):
            pl = pool if pool is not None else ps
            psy = pl.tile([128, 512], F32, name=f"psy{m}_{ns}",
                          tag="st" if pl is ps_st else "ps")
            for k in range(kmax):
                nc.tensor.matmul(
                    psy[:, :],
                    wp_sb[:, k * C + m * 128 : k * C + (m + 1) * 128],
                    ot_ap(k, 0, 128, ns * 512, (ns + 1) * 512),
                    start=(k == 0),
                    stop=False,
                )
            return psy

        def emit_proj_open(m, ns, kmax, pool=None):
            pl = pool if pool is not None else ps
            psy = pl.tile([128, 512], F32, name=f"psy{m}_{ns}",
                          tag="st" if pl is ps_st else "ps")
            for k in range(kmax):
                nc.tensor.matmul(
                    psy[:, :],
                    wp_sb[:, k * C + m * 128 : k * C + (m + 1) * 128],
                    ot_ap(k, 0, 128, ns * 512, (ns + 1) * 512),
                    start=(k == 0),
                    stop=False,
                )
            return psy

        def emit_proj_close(psy, m, ns, kmin):
            for k in range(kmin, KT):
                nc.tensor.matmul(
                    psy[:, :],
                    wp_sb[:, k * C + m * 128 : k * C + (m + 1) * 128],
                    ot_ap(k, 0, 128, ns * 512, (ns + 1) * 512),
                    start=False,
                    stop=(k == KT - 1),
                )
            yt = attn.tile([128, 512], F32, name=f"yt{m}_{ns}", tag="yt", bufs=3)
            nc.scalar.add(yt[:, :], psy[:, :], bp_sb[:, m : m + 1])
            eng = nc.scalar if m % 2 else nc.sync
            eng.dma_start(
                out=out_d[m * 128 : (m + 1) * 128, ns * 512 : (ns + 1) * 512],
                in_=yt[:, :],
            )

        def emit_proj(m, ns):
            emit_proj_close(emit_proj_open(m, ns, 5), m, ns, 5)

        def emit_qk_pair(m1, m2, n):
            """two k-interleaved qk groups — halves the DMA-paced prologue"""
            tiles = []
            for m in (m1, m2):
                bb, half = m % 6, (0 if m < 6 else 1)
                psq = ps.tile([128, 512], F32, name=f"psq{m}_{n}", tag="ps")
                tiles.append((m, bb, half, psq))
            for k in range(KT):
                for m, bb, half, psq in tiles:
                    nc.tensor.matmul(
                        psq[:, :],
                        wqp_sb[:, bb * KT * 256 + k * 256 + half * 128 :
                               bb * KT * 256 + k * 256 + half * 128 + 128],
                        xt_sb[:, k * NT + n * 512 : k * NT + (n + 1) * 512],
                        start=(k == 0),
                        stop=(k == KT - 1),
                    )
            for m, bb, half, psq in tiles:
                nc.vector.tensor_copy(
                    qk_sb[:, m * NT + n * 512 : m * NT + n * 512 + 512], psq[:, :]
                )

        # ---- prologue: qk for pair 0 (DMA-paced), then deferred DMAs ----
        for n in range(NQ):
            emit_qk_pair(6, 0, n)
        dma_v_cols()

        # ---- main loop over head pairs ----
        held = {}
        for b in range(6):
            fillers = []
            if b == 0:
                fillers.append(lambda: dma_qk_cols(1))
                for t in range(TT):
                    fillers.append(lambda t=t: emit_v_group(t, 0))
                fillers.insert(3, lambda: (emit_qk_group(1, 0),
                                           emit_qk_group(1, 1)))
                fillers.insert(6, lambda: (emit_qk_group(7, 0),
                                           emit_qk_group(7, 1)))
                fillers.append(dma_wproj)
            elif b < 5:
                fillers.append(lambda b=b: dma_qk_cols(b + 1))
                fillers.append(lambda b=b: (emit_qk_group(b + 1, 0),
                                            emit_qk_group(b + 1, 1)))
                fillers.append(lambda b=b: (emit_pv_norm(2 * b - 2, 0),
                                            emit_pv_norm(2 * b - 1, 0)))
                fillers.append(lambda b=b: (emit_qk_group(7 + b, 0),
                                            emit_qk_group(7 + b, 1)))
                fillers.append(lambda b=b: (emit_pv_norm(2 * b - 2, 1),
                                            emit_pv_norm(2 * b - 1, 1)))
                if b <= 2:
                    for tt2 in range(4 * (b - 1), 4 * b):
                        fillers.append(lambda tt2=tt2: emit_v_group(tt2, 1))
            else:
                fillers.append(lambda: (emit_pv_norm(8, 0), emit_pv_norm(9, 0)))
                fillers.append(lambda: held.update(g00=emit_proj_open(0, 0, 5)))
                fillers.append(lambda: (emit_pv_norm(8, 1), emit_pv_norm(9, 1)))
                fillers.append(lambda: held.update(g10=emit_proj_open(1, 0, 5)))
            pair_tiles[b] = attn.tile([128, TT * 2048], BF16, name=f"epair{b}",
                                      tag="epair", bufs=2)
            fi = 0
            for kt in range(TT):
                emit_st_pair_kt(b, kt)
                if fi < len(fillers):
                    fillers[fi]()
                    fi += 1
            while fi < len(fillers):
                fillers[fi]()
                fi += 1

        # ---- tail: PV/norm heads 10/11 interleaved with projection ----
        emit_pv_norm(10, 0)
        emit_pv_norm(11, 0)
        held["g20"] = emit_proj_open(2, 0, 5, pool=ps_st)
        emit_pv_norm(10, 1)
        held["g30"] = emit_proj_open(3, 0, 5, pool=ps_st)
        emit_pv_norm(11, 1)
        emit_proj_close(held["g00"], 0, 0, 5)
        emit_proj_close(held["g10"], 1, 0, 5)
        emit_proj_close(held["g20"], 2, 0, 5)
        emit_proj_close(held["g30"], 3, 0, 5)
        emit_proj(4, 0)
        emit_proj(5, 0)
        for m in range(KT):
            emit_proj(m, 1)


_NC = None


def build_nc():
    global _NC
    if _NC is None:
        nc = bacc.Bacc(
            trn_type="TRN2",
            target_bir_lowering=False,
            debug=False,
            enable_asserts=False,
            num_devices=8,
        )
        with tile.TileContext(nc) as tc:
            build_graph(tc)
        nc.compile()
        _NC = nc
    return _NC


def make_in_maps(x, w_qkv, w_proj, b_proj):
    import ml_dtypes

    bf16 = ml_dtypes.bfloat16
    x = np.asarray(x, dtype=np.float32)
    w_qkv = np.asarray(w_qkv, dtype=np.float32).astype(bf16)
    w_proj = np.ascontiguousarray(np.asarray(w_proj, dtype=np.float32).astype(bf16))
    b_proj = np.asarray(b_proj, dtype=np.float32)
    xT = np.ascontiguousarray(x.transpose(0, 2, 1).astype(bf16))  # [8, 768, 1024]
    # per-pair packed q/k col blocks in SBUF layout: [6, 128, KT*256]
    wqkp = np.empty((6, 128, KT * 256), dtype=bf16)
    for b in range(6):
        blk = np.concatenate(
            [w_qkv[:, b * 128 : (b + 1) * 128],
             w_qkv[:, 768 + b * 128 : 768 + (b + 1) * 128]], axis=1)  # [768, 256]
        wqkp[b] = np.ascontiguousarray(
            blk.reshape(KT, 128, 256).transpose(1, 0, 2).reshape(128, KT * 256))
    wv = np.ascontiguousarray(w_qkv[:, 1536:])                    # [768, 768]
    bp = np.ascontiguousarray(b_proj.reshape(KT, 128).T)          # [128, 6]
    return [
        {"xT": xT[i], "wqkp": wqkp, "wv": wv, "wproj": w_proj, "bproj": bp}
        for i in range(B)
    ]


def run_on_hw(in_maps, trace=False, **kwargs):
    from concourse.bass_utils import run_bass_kernel_spmd

    nc = build_nc()
    return run_bass_kernel_spmd(
        nc, in_maps, core_ids=list(range(B)), trace=trace, **kwargs
    )


def kernel(x, w_qkv, w_proj, b_proj):
    in_maps = make_in_maps(x, w_qkv, w_proj, b_proj)
    res = run_on_hw(in_maps, trace=False)
    out = np.stack([np.asarray(res.results[i]["out"]).T for i in range(B)])
    return np.ascontiguousarray(out.astype(np.float32))
